# revision 64
# baseline (speedup 1.0000x reference)
"""Trainium2 Bass kernel for nn_DiscriminationLoss (segment_reduce).

Math: the reference loss reduces to, per image b:
  S[b,k,c]    = sum of pred[b,c] over pixels with label k   (k=1..16 needed)
  counts[b,k] = histogram of labels                          (k=0..16)
followed by a tiny scalar epilogue on the host:
  N = ||S||_2 over c, N[0]=0; f = log(relu(3-N)^2+1)
  sum_g = counts . f; own/other/scale pair-combination; final scalar sum.

Device strategy (per core, 2 images, data-parallel over batch):
- Pixels live as [128 partitions, 6400 columns]; a "chunk" is one column
  (128 pixels), a "group" is 8 consecutive chunks.
- Mask slabs: per group a [128, 128] bf16 slab whose column (k-1)*8+i
  covers chunk 8g+i.  Planes are STEP masks, not one-hot: DVE planes
  (k < ACT_M) hold [lab >= k] via tensor_scalar(is_ge) (one op per k
  spanning a whole multi-group slab with a 3D strided output AP, which
  keeps the DVE 4x performance mode), and Act planes (k >= ACT_M) hold
  sign(lab - k + 0.5) = +/-1 -- a SINGLE activation op per plane (vs two
  for any 0/1 indicator), exact since the argument is never 0.  The host
  telescopes: S_k = S>=k - S>=k+1, with Act planes decoded as
  S>=k = (G_k + S>=1)/2, anchored by the DVE k=1 step plane and made
  exact by host-zeroing pred on background pixels (S_0 is never used).
  walrus rejects ALU ops on the Pool engine, so gpsimd only does memsets
  and the output DMA.  accum_out yields the histogram for free, decoded
  by the same telescope.  ACT_M is per image (14 / 13): the psum segments
  decode independently and one image carries an extra Act plane to balance
  the engines (assignment tuned by sweep).
- PE consumes each group with ONE matmul: stationary = mask slab [128,128]
  (LdWeights), moving = pred fp8 [128, 32] (8 chunks x 4 channels), PSUM
  [128, 32] accumulated over a whole accumulation segment.  Only the 8
  diagonal (chunk_i == chunk_j) blocks are meaningful; the host sums
  psum[(k-1)*8+i, i*4+c] over i to get S[b,k,c].  Off-diagonal products are
  computed-but-never-read garbage.  This amortizes the moving stream to
  4 columns per chunk -> ~13us PE vs ~43us for per-chunk matmuls.
- pred is fp8e4m3 (host-converted): segment sums only need to clear the
  relu(3-||S||) threshold with ||S|| ~ 300, so 4% element error is noise;
  halves DMA bytes vs bf16.  Labels are bf16 (exact for 0..16; 2-byte dtype
  needed for the DVE 4x mode).
- Slab sizes [64, 336 | 282, 118] keep the first compute start early and
  the tail short; image 1's PSUM closes in two banks (slab 2 / slab 3) so
  the big drain + main out-DMA descriptor generation overlap the PE tail.

Toolchain workarounds:
- walrus rejects instructions carrying more than one sync wait, and any
  wait riding tensor_scalar compute; the BIR is post-processed to move
  those waits onto same-engine single-wait Drain predecessors.

Sharding: data-parallel over batch, 2 images per core, no collectives.
"""

import json

import numpy as np
import ml_dtypes

import concourse.bass as bass
import concourse.mybir as mybir
import concourse.tile as tile
import concourse.bass2jax as _b2j
from concourse.bass_utils import run_bass_kernel_spmd


def _split_multiwait_bir(bir_json: bytes) -> bytes:
    """walrus in this container rejects instructions carrying more than one
    sync wait. Tile's kernel-tail drain aggregates one wait per DMA/engine
    sem lane onto a single SP Drain, so split any multi-wait instruction
    into single-wait predecessors on the same engine."""
    d = json.loads(bir_json)
    changed = False
    for fn in d.get("functions", []):
        for bb in fn.get("blocks", []):
            insts = bb.get("instructions", [])
            out = []
            for ins in insts:
                si = ins.get("sync_info") or {}
                waits = si.get("on_wait") or []
                # TensorScalarPtr cannot carry ANY wait; others only one
                keep = 0 if (waits and ins.get("opcode") == "TensorScalarPtr") \
                    else 1
                if len(waits) > keep:
                    changed = True
                    split = waits[:len(waits) - keep]
                    for wi, w in enumerate(split):
                        out.append(
                            {
                                "debug": ins.get("debug"),
                                "engine": ins["engine"],
                                "ins": [],
                                "is_reset_sema": False,
                                "name": f"{ins['name']}_w{wi}",
                                "opcode": "Drain",
                                "outs": [],
                                "sync_info": {"on_update": [], "on_wait": [w]},
                            }
                        )
                    si["on_wait"] = waits[len(waits) - keep:]
                out.append(ins)
            bb["instructions"] = out
    if not changed:
        return bir_json
    return json.dumps(d).encode()


_ORIG_COMPILE_BIR = _b2j.compile_bir_kernel


def _compile_bir_splitting_waits(bir_json, tmpdir, neff_name="file.neff"):
    return _ORIG_COMPILE_BIR(_split_multiwait_bir(bir_json), tmpdir, neff_name=neff_name)


_b2j.compile_bir_kernel = _compile_bir_splitting_waits

B, C, H, W = 16, 4, 640, 640
HW = H * W                 # 409600
P = 128
FD = HW // P               # 3200 columns per image
N_CORES = 8
IPC = B // N_CORES         # images per core
KMAX = 16
K1 = KMAX + 1
SIGMA_DIS = 3.0
F0 = float(np.log(SIGMA_DIS**2 + 1.0))

NCOL = IPC * FD            # 6400 pixel-columns per core
NG = NCOL // 8             # 800 groups of 8 chunks
# slab sizes in groups; must not cross the image boundary (group 400).
# small first slab -> compute starts early; small last slab -> short tail.
SLABS = [64, 336, 282, 118]
assert sum(SLABS) == NG and sum(SLABS[:2]) == NG // 2
NSLAB = len(SLABS)

# out layout: [128, 64 cnt | 32 psum img0 | 32 psum img1a | 32 psum img1b]
# f32.  Image 1's accumulation closes in two banks (slab 2 / slab 3) so the
# big drain and the main out-DMA descriptor generation overlap the PE tail.
CNT_COLS = NSLAB * KMAX    # 64: per-slab per-k partition histogram partials
NPS = 3
OUT_COLS = CNT_COLS + NPS * 32

# planes k >= ACT_M live on the Activation engine; per-image (the psum
# segments decode independently, and image 0's bigger Act share balances
# the engines better over its 400-group span)
ACT_MS = [14, 13]
ACT_MIN = min(ACT_MS)

# test.py can set RUN_KWARGS["trace"] = True and read LAST_RESULT for profiling
RUN_KWARGS = {}
LAST_RESULT = None
_NC_CACHE = []

BF16 = mybir.dt.bfloat16
FP8 = mybir.dt.float8e4
F32 = mybir.dt.float32
AF = mybir.ActivationFunctionType


def _build_nc():
    nc = bass.Bass("TRN2", target_bir_lowering=False, debug=False)
    pred_d = nc.dram_tensor("pred", [P, NCOL * C], FP8, kind="ExternalInput")
    lab_d = nc.dram_tensor("lab", [P, NCOL], BF16, kind="ExternalInput")
    out_d = nc.dram_tensor("out", [P, OUT_COLS], F32, kind="ExternalOutput")

    with tile.TileContext(nc) as tc:
        with tc.tile_pool(name="pool", bufs=1) as pool, \
             tc.tile_pool(name="ps", bufs=2, space="PSUM") as pspool:
            pred_sb = pool.tile([P, NCOL * C], FP8, name="pred_sb")
            lab_sb = pool.tile([P, NCOL], BF16, name="lab_sb")
            acc = pool.tile([P, OUT_COLS], F32, name="acc")
            # per-partition bias scalars for the Act-engine sign planes
            consts = pool.tile([P, 3 + K1 - ACT_MIN], F32, name="consts")
            for k in range(ACT_MIN, K1):
                nc.gpsimd.memset(consts[:, 3 + k - ACT_MIN:4 + k - ACT_MIN],
                                 0.5 - float(k))

            # per-slab input DMAs so compute starts as slices land; labels
            # are front-loaded (DVE consumes them first and is the critical
            # engine), pred interleaves behind
            bounds = []
            g0 = 0
            for gs in SLABS:
                bounds.append((g0, g0 + gs))
                g0 += gs
            nc.sync.dma_start(lab_sb[:, bounds[0][0] * 8:bounds[0][1] * 8],
                              lab_d[:, bounds[0][0] * 8:bounds[0][1] * 8])
            nc.sync.dma_start(lab_sb[:, bounds[1][0] * 8:bounds[1][1] * 8],
                              lab_d[:, bounds[1][0] * 8:bounds[1][1] * 8])
            nc.sync.dma_start(pred_sb[:, bounds[0][0] * 32:bounds[0][1] * 32],
                              pred_d[:, bounds[0][0] * 32:bounds[0][1] * 32])
            nc.sync.dma_start(lab_sb[:, bounds[2][0] * 8:bounds[3][1] * 8],
                              lab_d[:, bounds[2][0] * 8:bounds[3][1] * 8])
            for lo, hi in bounds[1:]:
                nc.sync.dma_start(pred_sb[:, lo * 32:hi * 32],
                                  pred_d[:, lo * 32:hi * 32])

            pred4 = pred_sb[:].rearrange("p (g m) -> p g m", m=32)  # [P, NG, 32]

            psum = [pspool.tile([P, 32], F32, name=f"ps_{i}") for i in range(NPS)]
            # accumulation segment boundaries (inclusive start, exclusive end)
            # and which psum tile each segment uses
            segs = [(0, NG // 2, 0), (NG // 2, NG - SLABS[-1], 1),
                    (NG - SLABS[-1], NG, 2)]

            def seg_of(g):
                for lo, hi, pi in segs:
                    if lo <= g < hi:
                        return lo, hi, pi
                raise AssertionError(g)
            g0 = 0
            for s, gs in enumerate(SLABS):
                img = (2 * g0) // NG
                slab = pool.tile([P, gs * 128], BF16, name=f"slab_{s}",
                                 tag="slab", bufs=2)
                slab3 = slab[:].rearrange("p (g m) -> p g m", m=128)
                lab3 = lab_sb[:, g0 * 8:(g0 + gs) * 8].rearrange(
                    "p (g i) -> p g i", i=8)
                # tensor_scalar ops may carry waits here: the BIR post-pass
                # moves every wait on a TensorScalarPtr onto same-engine
                # Drain predecessors, which walrus accepts
                # step masks [lab >= k] on DVE (k < ACT_M, plane slot k-1).
                # walrus only allows tensor_scalar on DVE (the Pool engine
                # fails its codegen check).  The host telescopes
                # S_k = S>=k - S>=k+1, so steps cost the same as equality
                # masks but let the Act engine build its planes in ONE op.
                act_m = ACT_MS[(2 * g0) // NG]
                for k in range(1, act_m):
                    nc.vector.tensor_scalar(
                        out=slab3[:, :, (k - 1) * 8:k * 8],
                        in0=lab3[:],
                        scalar1=float(k),
                        scalar2=None,
                        op0=mybir.AluOpType.is_ge,
                        op1=mybir.AluOpType.add,
                        accum_out=acc[:, s * KMAX + (k - 1):s * KMAX + k],
                    )
                # Act-engine sign planes: Sign(lab - k + 0.5) = +1 where
                # lab >= k else -1 (exact; the argument is never 0).  With
                # pred host-zeroed on background, G_k = 2*S>=k - S>=1 where
                # S>=1 comes from the DVE's k=1 step plane.  accum gives
                # A_k = 2*cnt>=k - NPIX.
                for k in range(act_m, K1):
                    nc.scalar.activation(
                        slab3[:, :, (k - 1) * 8:k * 8],
                        lab3[:],
                        AF.Sign, bias=consts[:, 3 + k - ACT_MIN:4 + k - ACT_MIN],
                        accum_out=acc[:, s * KMAX + (k - 1):s * KMAX + k],
                    )
                # absorber for the pred-DMA wait on the PE side: the first
                # matmul of each slab would otherwise carry the DMA sem wait
                # alongside its slab-ready wait.
                for gl in range(gs):
                    g = g0 + gl
                    lo, hi, pi = seg_of(g)
                    nc.tensor.matmul(
                        psum[pi][:],
                        slab3[:, gl, :],
                        pred4[:, g, :],
                        start=(g == lo),
                        stop=(g == hi - 1),
                    )
                g0 += gs

            for i in range(NPS):
                # gpsimd cannot read PSUM, so drains stay on DVE
                nc.vector.tensor_copy(
                    acc[:, CNT_COLS + i * 32:CNT_COLS + (i + 1) * 32], psum[i][:]
                )
            # main out-DMA (cnt + ps0 + ps1a) overlaps the last slab's
            # matmuls; only the tiny ps1b block ships after the PE finishes
            nc.gpsimd.dma_start(out_d[:, :OUT_COLS - 32], acc[:, :OUT_COLS - 32])
            nc.gpsimd.dma_start(out_d[:, OUT_COLS - 32:], acc[:, OUT_COLS - 32:])
    return nc


def _get_nc():
    if not _NC_CACHE:
        _NC_CACHE.append(_build_nc())
    return _NC_CACHE[0]


def make_in_maps(pred_similarities, kernel_labels):
    pred = np.ascontiguousarray(pred_similarities, dtype=np.float32).reshape(
        N_CORES, IPC, C, P, FD
    )
    labs0 = np.ascontiguousarray(kernel_labels, dtype=np.int32).reshape(
        N_CORES, IPC, P, FD
    )
    # zero pred on background pixels: S_0 is never used, every S_k (k>=1)
    # is unchanged, and it anchors the Act sign-plane decode via
    # G_k = 2*S>=k - S>=1  (no background leakage into the +/-1 sums)
    pred = pred * (labs0 != 0)[:, :, None, :, :]
    # fp8 e4m3 conversion; |pred| <= ~6 sigma so no saturation concerns
    pred8 = pred.astype(mybir.dt.np(FP8))
    # -> [cores, P, IPC, FD, C] so column t*4+c matches chunk-major layout
    pred8 = pred8.transpose(0, 3, 1, 4, 2).reshape(N_CORES, P, NCOL * C)

    labs = np.ascontiguousarray(kernel_labels, dtype=np.float32).reshape(
        N_CORES, IPC, P, FD
    )
    labs16 = labs.astype(ml_dtypes.bfloat16).transpose(0, 2, 1, 3).reshape(
        N_CORES, P, NCOL
    )
    return [
        {"pred": np.ascontiguousarray(pred8[i]), "lab": np.ascontiguousarray(labs16[i])}
        for i in range(N_CORES)
    ]


def kernel(pred_similarities, kernel_labels):
    global LAST_RESULT
    nc = _get_nc()
    in_maps = make_in_maps(pred_similarities, kernel_labels)
    res = run_bass_kernel_spmd(nc, in_maps, core_ids=list(range(N_CORES)), **RUN_KWARGS)
    LAST_RESULT = res
    outs = [np.asarray(res.results[c]["out"]) for c in range(N_CORES)]
    return epilogue(outs)


def epilogue(outs):
    S = np.zeros((B, K1, C), np.float64)
    counts = np.zeros((B, K1), np.float64)
    half = NSLAB // 2
    for core in range(N_CORES):
        o = outs[core].astype(np.float64)  # [P, OUT_COLS]
        for i in range(IPC):
            b = core * IPC + i
            # raw per-plane sums: DVE planes (k < ACT_M) hold >=-step data,
            # Act planes hold +/-1 sign data
            cnt = o[:, :CNT_COLS].reshape(P, NSLAB, KMAX)
            raw_c = cnt[:, i * half:(i + 1) * half, :].sum(axis=(0, 1))  # [16]
            raw_s = np.zeros((KMAX, C))
            blocks = [1, 2] if i == 1 else [0]
            for pi in blocks:
                ps = o[:, CNT_COLS + pi * 32:CNT_COLS + (pi + 1) * 32]
                ps4 = ps.reshape(KMAX, 8, 8, C)  # [k-1, i_row, i_col, c]
                raw_s += np.einsum("kiic->kc", ps4)
            # telescope: S>=k / cnt>=k, then difference
            act_m = ACT_MS[i]
            s_ge = np.zeros((K1 + 1, C))
            c_ge = np.zeros(K1 + 1)
            for k in range(1, act_m):
                s_ge[k] = raw_s[k - 1]
                c_ge[k] = raw_c[k - 1]
            for k in range(act_m, K1):
                s_ge[k] = (raw_s[k - 1] + s_ge[1]) / 2.0
                c_ge[k] = (raw_c[k - 1] + HW) / 2.0
            S[b, 1:, :] = s_ge[1:K1] - s_ge[2:K1 + 1]
            counts[b, 1:] = c_ge[1:K1] - c_ge[2:K1 + 1]
            counts[b, 0] = HW - c_ge[1]

    # scalar epilogue, mirroring reference.py
    N = np.linalg.norm(S, axis=-1)
    N[:, 0] = 0.0
    f = np.log(np.maximum(SIGMA_DIS - N, 0.0) ** 2 + 1.0)
    sum_g = (counts * f).sum(axis=-1)
    present = counts > 0
    Kb = np.where(
        present.any(axis=1), (present * np.arange(K1)).max(axis=1), 0
    ).astype(np.float64)
    active = Kb > 1.0
    Pn = Kb * (Kb - 1.0) * 0.5
    own = np.where(active, (Kb - 1.0) * sum_g + HW * (Pn - (Kb - 1.0)) * F0, 0.0)
    P_act = np.where(active, Pn, 0.0)
    other = (P_act.sum() - P_act) * HW * F0
    scale = np.where(active, 1.0 / (Kb * (Kb - 1.0)), Kb)
    return np.float32((scale * (own + other)).sum())


# revision 65
# speedup vs baseline: 1.0138x; 1.0138x over previous
"""Trainium2 Bass kernel for nn_DiscriminationLoss (segment_reduce).

Math: the reference loss reduces to, per image b:
  S[b,k,c]    = sum of pred[b,c] over pixels with label k   (k=1..16 needed)
  counts[b,k] = histogram of labels                          (k=0..16)
followed by a tiny scalar epilogue on the host:
  N = ||S||_2 over c, N[0]=0; f = log(relu(3-N)^2+1)
  sum_g = counts . f; own/other/scale pair-combination; final scalar sum.

Device strategy (per core, 2 images, data-parallel over batch):
- Pixels live as [128 partitions, 6400 columns]; a "chunk" is one column
  (128 pixels), a "group" is 8 consecutive chunks.
- Mask slabs: per group a [128, 128] bf16 slab whose column (k-1)*8+i
  covers chunk 8g+i.  Planes are STEP masks, not one-hot: DVE planes
  (k < ACT_M) hold [lab >= k] via tensor_scalar(is_ge) (one op per k
  spanning a whole multi-group slab with a 3D strided output AP, which
  keeps the DVE 4x performance mode), and Act planes (k >= ACT_M) hold
  sign(lab - k + 0.5) = +/-1 -- a SINGLE activation op per plane (vs two
  for any 0/1 indicator), exact since the argument is never 0.  The host
  telescopes: S_k = S>=k - S>=k+1, with Act planes decoded as
  S>=k = (G_k + S>=1)/2, anchored by the DVE k=1 step plane and made
  exact by host-zeroing pred on background pixels (S_0 is never used).
  walrus rejects ALU ops on the Pool engine, so gpsimd only does memsets
  and the output DMA.  accum_out yields the histogram for free, decoded
  by the same telescope.  ACT_M is per image (14 / 13): the psum segments
  decode independently and one image carries an extra Act plane to balance
  the engines (assignment tuned by sweep).
- PE consumes each group with ONE matmul: stationary = mask slab [128,128]
  (LdWeights), moving = pred fp8 [128, 32] (8 chunks x 4 channels), PSUM
  [128, 32] accumulated over a whole accumulation segment.  Only the 8
  diagonal (chunk_i == chunk_j) blocks are meaningful; the host sums
  psum[(k-1)*8+i, i*4+c] over i to get S[b,k,c].  Off-diagonal products are
  computed-but-never-read garbage.  This amortizes the moving stream to
  4 columns per chunk -> ~13us PE vs ~43us for per-chunk matmuls.
- pred is fp8e4m3 (host-converted): segment sums only need to clear the
  relu(3-||S||) threshold with ||S|| ~ 300, so 4% element error is noise;
  halves DMA bytes vs bf16.  Labels are bf16 (exact for 0..16; 2-byte dtype
  needed for the DVE 4x mode).
- Slab sizes [64, 336 | 282, 118] keep the first compute start early and
  the tail short; image 1's PSUM closes in two banks (slab 2 / slab 3) so
  the big drain + main out-DMA descriptor generation overlap the PE tail.

Toolchain workarounds:
- walrus rejects instructions carrying more than one sync wait, and any
  wait riding tensor_scalar compute; the BIR is post-processed to move
  those waits onto same-engine single-wait Drain predecessors.

Sharding: data-parallel over batch, 2 images per core, no collectives.
"""

import json

import numpy as np
import ml_dtypes

import concourse.bass as bass
import concourse.mybir as mybir
import concourse.tile as tile
import concourse.bass2jax as _b2j
from concourse.bass_utils import run_bass_kernel_spmd


def _split_multiwait_bir(bir_json: bytes) -> bytes:
    """walrus in this container rejects instructions carrying more than one
    sync wait. Tile's kernel-tail drain aggregates one wait per DMA/engine
    sem lane onto a single SP Drain, so split any multi-wait instruction
    into single-wait predecessors on the same engine."""
    d = json.loads(bir_json)
    changed = False
    for fn in d.get("functions", []):
        for bb in fn.get("blocks", []):
            insts = bb.get("instructions", [])
            out = []
            for ins in insts:
                si = ins.get("sync_info") or {}
                waits = si.get("on_wait") or []
                # TensorScalarPtr cannot carry ANY wait; others only one
                keep = 0 if (waits and ins.get("opcode") == "TensorScalarPtr") \
                    else 1
                if len(waits) > keep:
                    changed = True
                    split = waits[:len(waits) - keep]
                    for wi, w in enumerate(split):
                        out.append(
                            {
                                "debug": ins.get("debug"),
                                "engine": ins["engine"],
                                "ins": [],
                                "is_reset_sema": False,
                                "name": f"{ins['name']}_w{wi}",
                                "opcode": "Drain",
                                "outs": [],
                                "sync_info": {"on_update": [], "on_wait": [w]},
                            }
                        )
                    si["on_wait"] = waits[len(waits) - keep:]
                out.append(ins)
            bb["instructions"] = out
    if not changed:
        return bir_json
    return json.dumps(d).encode()


_ORIG_COMPILE_BIR = _b2j.compile_bir_kernel


def _compile_bir_splitting_waits(bir_json, tmpdir, neff_name="file.neff"):
    return _ORIG_COMPILE_BIR(_split_multiwait_bir(bir_json), tmpdir, neff_name=neff_name)


_b2j.compile_bir_kernel = _compile_bir_splitting_waits

B, C, H, W = 16, 4, 640, 640
HW = H * W                 # 409600
P = 128
FD = HW // P               # 3200 columns per image
N_CORES = 8
IPC = B // N_CORES         # images per core
KMAX = 16
K1 = KMAX + 1
SIGMA_DIS = 3.0
F0 = float(np.log(SIGMA_DIS**2 + 1.0))

NCOL = IPC * FD            # 6400 pixel-columns per core
NG = NCOL // 8             # 800 groups of 8 chunks
# slab sizes in groups; must not cross the image boundary (group 400).
# small first slab -> compute starts early; small last slab -> short tail.
SLABS = [64, 336, 282, 118]
assert sum(SLABS) == NG and sum(SLABS[:2]) == NG // 2
NSLAB = len(SLABS)

# out layout: [128, 64 cnt | 32 psum img0 | 32 psum img1a | 32 psum img1b]
# f32.  Image 1's accumulation closes in two banks (slab 2 / slab 3) so the
# big drain and the main out-DMA descriptor generation overlap the PE tail.
CNT_COLS = NSLAB * KMAX    # 64: per-slab per-k partition histogram partials
NPS = 3
OUT_COLS = CNT_COLS + NPS * 32

# planes k >= ACT_M live on the Activation engine; per-image (the psum
# segments decode independently, and image 0's bigger Act share balances
# the engines better over its 400-group span)
ACT_MS = [14, 13]
ACT_MIN = min(ACT_MS)

# test.py can set RUN_KWARGS["trace"] = True and read LAST_RESULT for profiling
RUN_KWARGS = {}
LAST_RESULT = None
_NC_CACHE = []

BF16 = mybir.dt.bfloat16
FP8 = mybir.dt.float8e4
F32 = mybir.dt.float32
AF = mybir.ActivationFunctionType


def _build_nc():
    nc = bass.Bass("TRN2", target_bir_lowering=False, debug=False)
    pred_d = nc.dram_tensor("pred", [P, NCOL * C], FP8, kind="ExternalInput")
    lab_d = nc.dram_tensor("lab", [P, NCOL], BF16, kind="ExternalInput")
    out_d = nc.dram_tensor("out", [P, OUT_COLS], F32, kind="ExternalOutput")

    with tile.TileContext(nc) as tc:
        with tc.tile_pool(name="pool", bufs=1) as pool, \
             tc.tile_pool(name="ps", bufs=2, space="PSUM") as pspool:
            pred_sb = pool.tile([P, NCOL * C], FP8, name="pred_sb")
            lab_sb = pool.tile([P, NCOL], BF16, name="lab_sb")
            acc = pool.tile([P, OUT_COLS], F32, name="acc")
            # per-partition bias scalars for the Act-engine sign planes
            consts = pool.tile([P, 3 + K1 - ACT_MIN], F32, name="consts")
            for k in range(ACT_MIN, K1):
                nc.gpsimd.memset(consts[:, 3 + k - ACT_MIN:4 + k - ACT_MIN],
                                 0.5 - float(k))

            # per-slab input DMAs so compute starts as slices land; labels
            # are front-loaded (DVE consumes them first and is the critical
            # engine), pred interleaves behind
            bounds = []
            g0 = 0
            for gs in SLABS:
                bounds.append((g0, g0 + gs))
                g0 += gs
            nc.sync.dma_start(lab_sb[:, bounds[0][0] * 8:bounds[0][1] * 8],
                              lab_d[:, bounds[0][0] * 8:bounds[0][1] * 8])
            nc.sync.dma_start(lab_sb[:, bounds[1][0] * 8:bounds[1][1] * 8],
                              lab_d[:, bounds[1][0] * 8:bounds[1][1] * 8])
            nc.sync.dma_start(pred_sb[:, bounds[0][0] * 32:bounds[0][1] * 32],
                              pred_d[:, bounds[0][0] * 32:bounds[0][1] * 32])
            nc.sync.dma_start(lab_sb[:, bounds[2][0] * 8:bounds[3][1] * 8],
                              lab_d[:, bounds[2][0] * 8:bounds[3][1] * 8])
            for lo, hi in bounds[1:]:
                nc.sync.dma_start(pred_sb[:, lo * 32:hi * 32],
                                  pred_d[:, lo * 32:hi * 32])

            pred4 = pred_sb[:].rearrange("p (g m) -> p g m", m=32)  # [P, NG, 32]

            psum = [pspool.tile([P, 32], F32, name=f"ps_{i}") for i in range(NPS)]
            # accumulation segment boundaries (inclusive start, exclusive end)
            # and which psum tile each segment uses
            segs = [(0, NG // 2, 0), (NG // 2, NG - SLABS[-1], 1),
                    (NG - SLABS[-1], NG, 2)]

            def seg_of(g):
                for lo, hi, pi in segs:
                    if lo <= g < hi:
                        return lo, hi, pi
                raise AssertionError(g)
            g0 = 0
            for s, gs in enumerate(SLABS):
                img = (2 * g0) // NG
                slab = pool.tile([P, gs * 128], BF16, name=f"slab_{s}",
                                 tag="slab", bufs=2)
                slab3 = slab[:].rearrange("p (g m) -> p g m", m=128)
                lab3 = lab_sb[:, g0 * 8:(g0 + gs) * 8].rearrange(
                    "p (g i) -> p g i", i=8)
                # tensor_scalar ops may carry waits here: the BIR post-pass
                # moves every wait on a TensorScalarPtr onto same-engine
                # Drain predecessors, which walrus accepts
                # step masks [lab >= k] on DVE (k < ACT_M, plane slot k-1).
                # walrus only allows tensor_scalar on DVE (the Pool engine
                # fails its codegen check).  The host telescopes
                # S_k = S>=k - S>=k+1, so steps cost the same as equality
                # masks but let the Act engine build its planes in ONE op.
                act_m = ACT_MS[(2 * g0) // NG]
                for k in range(1, act_m):
                    nc.vector.tensor_scalar(
                        out=slab3[:, :, (k - 1) * 8:k * 8],
                        in0=lab3[:],
                        scalar1=float(k),
                        scalar2=None,
                        op0=mybir.AluOpType.is_ge,
                        op1=mybir.AluOpType.add,
                        accum_out=acc[:, s * KMAX + (k - 1):s * KMAX + k],
                    )
                # Act-engine sign planes: Sign(lab - k + 0.5) = +1 where
                # lab >= k else -1 (exact; the argument is never 0).  With
                # pred host-zeroed on background, G_k = 2*S>=k - S>=1 where
                # S>=1 comes from the DVE's k=1 step plane.  accum gives
                # A_k = 2*cnt>=k - NPIX.
                for k in range(act_m, K1):
                    nc.scalar.activation(
                        slab3[:, :, (k - 1) * 8:k * 8],
                        lab3[:],
                        AF.Sign, bias=consts[:, 3 + k - ACT_MIN:4 + k - ACT_MIN],
                        accum_out=acc[:, s * KMAX + (k - 1):s * KMAX + k],
                    )
                # absorber for the pred-DMA wait on the PE side: the first
                # matmul of each slab would otherwise carry the DMA sem wait
                # alongside its slab-ready wait.
                for gl in range(gs):
                    g = g0 + gl
                    lo, hi, pi = seg_of(g)
                    nc.tensor.matmul(
                        psum[pi][:],
                        slab3[:, gl, :],
                        pred4[:, g, :],
                        start=(g == lo),
                        stop=(g == hi - 1),
                    )
                g0 += gs

            for i in range(NPS):
                # gpsimd cannot read PSUM, so drains stay on DVE
                nc.vector.tensor_copy(
                    acc[:, CNT_COLS + i * 32:CNT_COLS + (i + 1) * 32], psum[i][:]
                )
            # main out-DMA (cnt + ps0 + ps1a) overlaps the last slab's
            # matmuls; only the tiny ps1b block ships after the PE finishes.
            # It goes via SP/HWDGE so its descriptor generation runs in
            # parallel with the main DMA's SWDGE generation on Pool.
            nc.gpsimd.dma_start(out_d[:, :OUT_COLS - 32], acc[:, :OUT_COLS - 32])
            nc.sync.dma_start(out_d[:, OUT_COLS - 32:], acc[:, OUT_COLS - 32:])
    return nc


def _get_nc():
    if not _NC_CACHE:
        _NC_CACHE.append(_build_nc())
    return _NC_CACHE[0]


def make_in_maps(pred_similarities, kernel_labels):
    pred = np.ascontiguousarray(pred_similarities, dtype=np.float32).reshape(
        N_CORES, IPC, C, P, FD
    )
    labs0 = np.ascontiguousarray(kernel_labels, dtype=np.int32).reshape(
        N_CORES, IPC, P, FD
    )
    # zero pred on background pixels: S_0 is never used, every S_k (k>=1)
    # is unchanged, and it anchors the Act sign-plane decode via
    # G_k = 2*S>=k - S>=1  (no background leakage into the +/-1 sums)
    pred = pred * (labs0 != 0)[:, :, None, :, :]
    # fp8 e4m3 conversion; |pred| <= ~6 sigma so no saturation concerns
    pred8 = pred.astype(mybir.dt.np(FP8))
    # -> [cores, P, IPC, FD, C] so column t*4+c matches chunk-major layout
    pred8 = pred8.transpose(0, 3, 1, 4, 2).reshape(N_CORES, P, NCOL * C)

    labs = np.ascontiguousarray(kernel_labels, dtype=np.float32).reshape(
        N_CORES, IPC, P, FD
    )
    labs16 = labs.astype(ml_dtypes.bfloat16).transpose(0, 2, 1, 3).reshape(
        N_CORES, P, NCOL
    )
    return [
        {"pred": np.ascontiguousarray(pred8[i]), "lab": np.ascontiguousarray(labs16[i])}
        for i in range(N_CORES)
    ]


def kernel(pred_similarities, kernel_labels):
    global LAST_RESULT
    nc = _get_nc()
    in_maps = make_in_maps(pred_similarities, kernel_labels)
    res = run_bass_kernel_spmd(nc, in_maps, core_ids=list(range(N_CORES)), **RUN_KWARGS)
    LAST_RESULT = res
    outs = [np.asarray(res.results[c]["out"]) for c in range(N_CORES)]
    return epilogue(outs)


def epilogue(outs):
    S = np.zeros((B, K1, C), np.float64)
    counts = np.zeros((B, K1), np.float64)
    half = NSLAB // 2
    for core in range(N_CORES):
        o = outs[core].astype(np.float64)  # [P, OUT_COLS]
        for i in range(IPC):
            b = core * IPC + i
            # raw per-plane sums: DVE planes (k < ACT_M) hold >=-step data,
            # Act planes hold +/-1 sign data
            cnt = o[:, :CNT_COLS].reshape(P, NSLAB, KMAX)
            raw_c = cnt[:, i * half:(i + 1) * half, :].sum(axis=(0, 1))  # [16]
            raw_s = np.zeros((KMAX, C))
            blocks = [1, 2] if i == 1 else [0]
            for pi in blocks:
                ps = o[:, CNT_COLS + pi * 32:CNT_COLS + (pi + 1) * 32]
                ps4 = ps.reshape(KMAX, 8, 8, C)  # [k-1, i_row, i_col, c]
                raw_s += np.einsum("kiic->kc", ps4)
            # telescope: S>=k / cnt>=k, then difference
            act_m = ACT_MS[i]
            s_ge = np.zeros((K1 + 1, C))
            c_ge = np.zeros(K1 + 1)
            for k in range(1, act_m):
                s_ge[k] = raw_s[k - 1]
                c_ge[k] = raw_c[k - 1]
            for k in range(act_m, K1):
                s_ge[k] = (raw_s[k - 1] + s_ge[1]) / 2.0
                c_ge[k] = (raw_c[k - 1] + HW) / 2.0
            S[b, 1:, :] = s_ge[1:K1] - s_ge[2:K1 + 1]
            counts[b, 1:] = c_ge[1:K1] - c_ge[2:K1 + 1]
            counts[b, 0] = HW - c_ge[1]

    # scalar epilogue, mirroring reference.py
    N = np.linalg.norm(S, axis=-1)
    N[:, 0] = 0.0
    f = np.log(np.maximum(SIGMA_DIS - N, 0.0) ** 2 + 1.0)
    sum_g = (counts * f).sum(axis=-1)
    present = counts > 0
    Kb = np.where(
        present.any(axis=1), (present * np.arange(K1)).max(axis=1), 0
    ).astype(np.float64)
    active = Kb > 1.0
    Pn = Kb * (Kb - 1.0) * 0.5
    own = np.where(active, (Kb - 1.0) * sum_g + HW * (Pn - (Kb - 1.0)) * F0, 0.0)
    P_act = np.where(active, Pn, 0.0)
    other = (P_act.sum() - P_act) * HW * F0
    scale = np.where(active, 1.0 / (Kb * (Kb - 1.0)), Kb)
    return np.float32((scale * (own + other)).sum())
